# revision 1
# baseline (speedup 1.0000x reference)
"""Sparse-attention Trainium2 kernel (8 NeuronCores, SPMD).

Sharding: core = b*4 + q handles batch b, image rows [28q, 28q+28) (3136 pixels).
Launch 1 pools x (7x7 avg -> 16x16 cells) and ctx (16x16 avg -> 7x7) on disjoint
row slices; host concatenates the tiny pooled partials. Launch 2 runs the full
attention pipeline per core on its 3136 pixels.
"""
import sys
sys.path.insert(0, "/opt/trn_rl_repo")
import numpy as np
import concourse.bacc as bacc
import concourse.mybir as mybir
from concourse.tile import TileContext
from concourse.bass_utils import run_bass_kernel_spmd

F32 = mybir.dt.float32
BF16 = mybir.dt.bfloat16
Copy = mybir.ActivationFunctionType.Copy
Exp = mybir.ActivationFunctionType.Exp
X = mybir.AxisListType.X
ALU = mybir.AluOpType

B, C, CC, H, W = 2, 768, 384, 112, 112
G, HD = 12, 64
S2, K2 = 49, 256
N = 28 * 112          # pixels per core
NW = 448              # window width (7 windows)
NSUB = 112            # stationary subtile width (4 per window)
NEG_INF = -1e30


def _new_nc():
    return bacc.Bacc("TRN2", target_bir_lowering=False, debug=False,
                     enable_asserts=False, num_devices=8)


def _build_pool_kernel():
    nc = _new_nc()
    x_sl = nc.dram_tensor("x_sl", [C, N], F32, kind="ExternalInput")
    ctx_sl = nc.dram_tensor("ctx_sl", [CC, 32 * 112], F32, kind="ExternalInput")
    vp = nc.dram_tensor("vp", [C, 64], F32, kind="ExternalOutput")
    cp = nc.dram_tensor("cp", [CC, 14], F32, kind="ExternalOutput")
    with TileContext(nc) as tc:
        with tc.tile_pool(name="sb", bufs=2) as sb:
            for t in range(6):
                xt = sb.tile([128, N], F32, tag="xt")
                nc.sync.dma_start(xt[:], x_sl[128 * t:128 * (t + 1), :])
                t1 = sb.tile([128, 28, 16], F32, tag="t1")
                nc.vector.tensor_reduce(
                    t1[:], xt[:].rearrange("p (h wb wi) -> p h wb wi", h=28, wi=7),
                    axis=X, op=ALU.add)
                t2 = sb.tile([128, 4, 16], F32, tag="t2")
                nc.vector.tensor_reduce(
                    t2[:], t1[:].rearrange("p (hb hi) w -> p hb w hi", hi=7),
                    axis=X, op=ALU.add)
                nc.sync.dma_start(vp[128 * t:128 * (t + 1), :],
                                  t2[:].rearrange("p a b -> p (a b)"))
            for t in range(3):
                ct = sb.tile([128, 32 * 112], F32, tag="ct")
                nc.sync.dma_start(ct[:], ctx_sl[128 * t:128 * (t + 1), :])
                u1 = sb.tile([128, 32, 7], F32, tag="u1")
                nc.vector.tensor_reduce(
                    u1[:], ct[:].rearrange("p (h wb wi) -> p h wb wi", h=32, wi=16),
                    axis=X, op=ALU.add)
                u2 = sb.tile([128, 2, 7], F32, tag="u2")
                nc.vector.tensor_reduce(
                    u2[:], u1[:].rearrange("p (hb hi) w -> p hb w hi", hi=16),
                    axis=X, op=ALU.add)
                nc.sync.dma_start(cp[128 * t:128 * (t + 1), :],
                                  u2[:].rearrange("p a b -> p (a b)"))
    nc.compile()
    return nc


def _build_main_kernel():
    nc = _new_nc()
    x_sl = nc.dram_tensor("x_sl", [C, N], F32, kind="ExternalInput")
    wqT = nc.dram_tensor("wqT", [C, C], F32, kind="ExternalInput")
    wkT = nc.dram_tensor("wkT", [CC, C], F32, kind="ExternalInput")
    wdT = nc.dram_tensor("wdT", [S2, K2], F32, kind="ExternalInput")
    pwT = nc.dram_tensor("pwT", [C, C], F32, kind="ExternalInput")
    pbm = nc.dram_tensor("pbm", [128, 6], F32, kind="ExternalInput")
    v_t = nc.dram_tensor("v_t", [K2, C], F32, kind="ExternalInput")
    cpool = nc.dram_tensor("cpool", [CC, S2], F32, kind="ExternalInput")
    ident = nc.dram_tensor("ident", [NSUB, NSUB], F32, kind="ExternalInput")
    sel2 = nc.dram_tensor("sel2", [2, 128], F32, kind="ExternalInput")
    out = nc.dram_tensor("out", [C, N], F32, kind="ExternalOutput")

    with TileContext(nc) as tc:
        with tc.tile_pool(name="wts", bufs=1) as wts, \
             tc.tile_pool(name="sb", bufs=2) as sb, \
             tc.tile_pool(name="ps", bufs=7, space="PSUM") as ps:

            def cast_in(dram, shape, tag, scale=None):
                f = sb.tile(shape, F32, tag="stage")
                nc.sync.dma_start(f[:], dram)
                b = wts.tile(shape, BF16, tag=tag)
                if scale is None:
                    nc.vector.tensor_copy(b[:], f[:])
                else:
                    nc.scalar.activation(b[:], f[:], Copy, scale=scale)
                return b

            # persistent weights (bf16)
            wq_bf = [cast_in(wqT[128 * t:128 * (t + 1), :], [128, C], f"wq{t}") for t in range(6)]
            pw_bf = [cast_in(pwT[128 * t:128 * (t + 1), :], [128, C], f"pw{t}") for t in range(6)]
            wd_bf = cast_in(wdT[:], [S2, K2], "wd")
            wd_hi_full = wts.tile([128, K2], BF16, tag="wdhi")
            nc.vector.tensor_copy(wd_hi_full[64:64 + S2, :], wd_bf[:])
            wd_hi = wd_hi_full[64:64 + S2, :]
            id_bf = cast_in(ident[:], [NSUB, NSUB], "id")
            s2_bf = cast_in(sel2[:], [2, 128], "s2")
            pb_sb = wts.tile([128, 6], F32, tag="pb")
            nc.sync.dma_start(pb_sb[:], pbm[:])

            # k = wk @ (ctx_pool/49): [768, 49] bf16, then block-diag pairs k2 [128, 98]
            cp_bf = [cast_in(cpool[128 * t:128 * (t + 1), :], [128, S2], f"cp{t}",
                             scale=1.0 / 49.0) for t in range(3)]
            wk_bf = [cast_in(wkT[128 * t:128 * (t + 1), :], [128, C], f"wk{t}") for t in range(3)]
            k2_bf = []
            for o in range(6):
                pk = ps.tile([128, S2], F32, tag="ps")
                for ci in range(3):
                    nc.tensor.matmul(pk[:], wk_bf[ci][:, 128 * o:128 * (o + 1)],
                                     cp_bf[ci][:], start=(ci == 0), stop=(ci == 2))
                kb = wts.tile([128, S2], BF16, tag=f"k{o}")
                nc.scalar.activation(kb[:], pk[:], Copy)
                k2 = wts.tile([128, 2 * S2], BF16, tag=f"k2_{o}")
                nc.vector.memset(k2[:], 0.0)
                nc.vector.tensor_copy(k2[0:64, 0:S2], kb[0:64, :])
                nc.vector.tensor_copy(k2[64:128, S2:2 * S2], kb[64:128, :])
                k2_bf.append(k2)

            # v_aug [128, 12, 65] per k-half: cols 0:64 v^T/49 per head, col 64 ones
            v_aug = []
            for kh in range(2):
                vb = cast_in(v_t[128 * kh:128 * (kh + 1), :], [128, C], f"v{kh}",
                             scale=1.0 / 49.0)
                va = wts.tile([128, G, 65], BF16, tag=f"va{kh}")
                nc.vector.memset(va[:], 1.0)
                nc.vector.tensor_copy(
                    va[:].rearrange("p g a -> p g a")[:, :, 0:64],
                    vb[:].rearrange("p (g a) -> p g a", a=64))
                v_aug.append(va)

            for w in range(7):
                cw = slice(NW * w, NW * (w + 1))
                # x window + bf16 cast + x+pb precompute
                x_f, x_bf, xpb = [], [], []
                for t in range(6):
                    xf = sb.tile([128, NW], F32, tag=f"x{t}")
                    nc.sync.dma_start(xf[:], x_sl[128 * t:128 * (t + 1), cw])
                    xb = sb.tile([128, NW], BF16, tag=f"xb{t}")
                    nc.vector.tensor_copy(xb[:], xf[:])
                    xp = sb.tile([128, NW], F32, tag=f"xp{t}")
                    nc.vector.tensor_scalar(xp[:], xf[:], pb_sb[:, t:t + 1], None,
                                            op0=ALU.add)
                    x_f.append(xf); x_bf.append(xb); xpb.append(xp)

                # q projection (scaled by hd^-0.5 on eviction)
                q_bf = []
                for o in range(6):
                    pq = ps.tile([128, NW], F32, tag="ps")
                    for ci in range(6):
                        nc.tensor.matmul(pq[:], wq_bf[ci][:, 128 * o:128 * (o + 1)],
                                         x_bf[ci][:], start=(ci == 0), stop=(ci == 5))
                    qb = sb.tile([128, NW], BF16, tag=f"q{o}")
                    nc.scalar.activation(qb[:], pq[:], Copy, scale=float(HD) ** -0.5)
                    q_bf.append(qb)

                oa_bf = []
                for t in range(6):
                    # attention scores for head pair t: [112, 4, 2, 49]
                    pa = ps.tile([NSUB, 4, 2 * S2], F32, tag="ps")
                    for s in range(4):
                        nc.tensor.matmul(pa[:, s, :],
                                         q_bf[t][:, NSUB * s:NSUB * (s + 1)],
                                         k2_bf[t][:])
                    at = sb.tile([NSUB, 4, 2, S2], F32, tag="at")
                    nc.scalar.activation(at[:], pa[:].rearrange("p a (b s) -> p a b s", s=S2), Copy)

                    # top-32 of 49: find 17th-smallest threshold via negated max8
                    ng = sb.tile([NSUB, 4, 2, S2], F32, tag="ng")
                    nc.vector.tensor_scalar_mul(ng[:], at[:], -1.0)
                    for s in range(4):
                        for hh in range(2):
                            m8 = sb.tile([NSUB, 8], F32, tag="m8")
                            sl = ng[:, s, hh, :]
                            nc.vector.max(out=m8[:], in_=sl)
                            nc.vector.match_replace(out=sl, in_to_replace=m8[:],
                                                    in_values=sl, imm_value=NEG_INF)
                            m8b = sb.tile([NSUB, 8], F32, tag="m8")
                            nc.vector.max(out=m8b[:], in_=sl)
                            nc.vector.match_replace(out=sl, in_to_replace=m8b[:],
                                                    in_values=sl, imm_value=NEG_INF)
                    tau = sb.tile([NSUB, 8], F32, tag="tau")
                    nc.vector.tensor_reduce(tau[:], ng[:].rearrange("p a b s -> p (a b) s"),
                                            axis=X, op=ALU.max)
                    nthr = sb.tile([NSUB, 8], F32, tag="nthr")
                    nc.vector.tensor_scalar_mul(nthr[:], tau[:], -1.0)
                    msk = sb.tile([NSUB, 4, 2, S2], F32, tag="msk")
                    nc.vector.tensor_tensor(
                        out=msk[:].rearrange("p a b s -> p (a b) s"),
                        in0=at[:].rearrange("p a b s -> p (a b) s"),
                        in1=nthr[:].to_broadcast((NSUB, 8, S2)), op=ALU.is_gt)
                    # sparse (bf16) into zero-padded [112, 4, 2, 64] for transpose
                    sp = sb.tile([NSUB, 4, 2, 64], BF16, tag="sp")
                    nc.gpsimd.memset(sp[:], 0.0)
                    nc.vector.tensor_tensor(out=sp[:, :, :, 0:S2], in0=at[:], in1=msk[:],
                                            op=ALU.mult)

                    # transpose via identity matmul -> spT [128, 4, 112] bf16
                    pt = ps.tile([128, 4, NSUB], F32, tag="ps")
                    for s in range(4):
                        nc.tensor.matmul(pt[:, s, :],
                                         sp[:, s, :, :].rearrange("p a b -> p (a b)"),
                                         id_bf[:])
                    spT = sb.tile([128, 4, NSUB], BF16, tag="spT")
                    nc.scalar.activation(spT[:], pt[:], Copy)

                    # per-head: omega logits -> exp -> out-mm (with sum row)
                    pouts = []
                    for hh in range(2):
                        base = 64 * hh
                        rhs = spT[base:base + S2, :, :].rearrange("p a b -> p (a b)")
                        po = ps.tile([65, NW], F32, tag="ps")
                        wd_use = wd_bf if hh == 0 else wd_hi
                        for kh in range(2):
                            pm = ps.tile([128, NW], F32, tag="ps")
                            nc.tensor.matmul(pm[:], wd_use[:, 128 * kh:128 * (kh + 1)], rhs)
                            ex = sb.tile([128, NW], BF16, tag="ex")
                            nc.scalar.activation(ex[:], pm[:], Exp)
                            nc.tensor.matmul(po[:], v_aug[kh][:, 2 * t + hh, :], ex[:],
                                             start=(kh == 0), stop=(kh == 1))
                        pouts.append(po)

                    # softmax denominators for the pair -> recip -> replicate
                    sm = sb.tile([2, NW], F32, tag="sm")
                    nc.scalar.activation(sm[0:1, :], pouts[0][64:65, :], Copy)
                    smt = sb.tile([1, NW], F32, tag="smt")
                    nc.scalar.activation(smt[0:1, :], pouts[1][64:65, :], Copy)
                    nc.sync.dma_start(sm[1:2, :], smt[0:1, :])
                    rc = sb.tile([2, NW], F32, tag="rc")
                    nc.vector.reciprocal(rc[:], sm[:])
                    rcb = sb.tile([2, NW], BF16, tag="rcb")
                    nc.vector.tensor_copy(rcb[:], rc[:])
                    pr = ps.tile([128, NW], F32, tag="ps")
                    nc.tensor.matmul(pr[:], s2_bf[:], rcb[:])
                    rep = sb.tile([128, NW], F32, tag="rep")
                    nc.scalar.activation(rep[:], pr[:], Copy)

                    oa = sb.tile([128, NW], BF16, tag=f"oa{t}")
                    for hh in range(2):
                        nc.vector.tensor_tensor(out=oa[64 * hh:64 * (hh + 1), :],
                                                in0=pouts[hh][0:64, :],
                                                in1=rep[64 * hh:64 * (hh + 1), :],
                                                op=ALU.mult)
                    oa_bf.append(oa)

                # output projection + bias + residual
                for o in range(6):
                    py = ps.tile([128, NW], F32, tag="ps")
                    for ci in range(6):
                        nc.tensor.matmul(py[:], pw_bf[ci][:, 128 * o:128 * (o + 1)],
                                         oa_bf[ci][:], start=(ci == 0), stop=(ci == 5))
                    y = sb.tile([128, NW], F32, tag=f"y{o}")
                    nc.vector.tensor_tensor(out=y[:], in0=py[:], in1=xpb[o][:], op=ALU.add)
                    nc.sync.dma_start(out[128 * o:128 * (o + 1), cw], y[:])
    nc.compile()
    return nc


_CACHE = {}


def kernel(x, ctx, wq, wk, wd, proj_w, proj_b):
    x = np.ascontiguousarray(np.asarray(x, dtype=np.float32))
    ctx = np.ascontiguousarray(np.asarray(ctx, dtype=np.float32))

    if "pool" not in _CACHE:
        _CACHE["pool"] = _build_pool_kernel()
    if "main" not in _CACHE:
        _CACHE["main"] = _build_main_kernel()

    # ---- launch 1: pooling partials
    ctx_starts = [0, 32, 64, 80]
    in1 = []
    for core in range(8):
        b, q = core // 4, core % 4
        xs = x[b, :, 28 * q:28 * (q + 1), :].reshape(C, N)
        cs = ctx[b, :, ctx_starts[q]:ctx_starts[q] + 32, :].reshape(CC, 32 * 112)
        in1.append({"x_sl": np.ascontiguousarray(xs), "ctx_sl": np.ascontiguousarray(cs)})
    r1 = run_bass_kernel_spmd(_CACHE["pool"], in1, list(range(8))).results

    v = np.zeros((B, C, 16, 16), np.float32)
    cpool = np.zeros((B, CC, 7, 7), np.float32)
    for core in range(8):
        b, q = core // 4, core % 4
        v[b, :, 4 * q:4 * (q + 1), :] = r1[core]["vp"].reshape(C, 4, 16)
        cpb = r1[core]["cp"].reshape(CC, 2, 7)
        if q < 3:
            cpool[b, :, 2 * q:2 * q + 2, :] = cpb
        else:
            cpool[b, :, 6, :] = cpb[:, 1, :]

    # ---- launch 2: main pipeline
    wqT = np.ascontiguousarray(wq.T.astype(np.float32))
    wkT = np.ascontiguousarray(wk.T.astype(np.float32))
    wdT = np.ascontiguousarray(wd.T.astype(np.float32))
    pwT = np.ascontiguousarray(proj_w.T.astype(np.float32))
    pbm = np.ascontiguousarray(proj_b.astype(np.float32).reshape(6, 128).T)
    ident = np.eye(NSUB, dtype=np.float32)
    sel2 = np.zeros((2, 128), np.float32)
    sel2[0, :64] = 1.0
    sel2[1, 64:] = 1.0

    in2 = []
    for core in range(8):
        b, q = core // 4, core % 4
        xs = np.ascontiguousarray(x[b, :, 28 * q:28 * (q + 1), :].reshape(C, N))
        vt = np.ascontiguousarray(v[b].reshape(C, K2).T)
        cpl = np.ascontiguousarray(cpool[b].reshape(CC, S2))
        in2.append({"x_sl": xs, "wqT": wqT, "wkT": wkT, "wdT": wdT, "pwT": pwT,
                    "pbm": pbm, "v_t": vt, "cpool": cpl, "ident": ident, "sel2": sel2})
    _CACHE["last_in2"] = in2
    r2 = run_bass_kernel_spmd(_CACHE["main"], in2, list(range(8))).results

    y = np.zeros((B, C, H, W), np.float32)
    for core in range(8):
        b, q = core // 4, core % 4
        y[b, :, 28 * q:28 * (q + 1), :] = r2[core]["out"].reshape(C, 28, 112)
    return y



# revision 8
# speedup vs baseline: 6.6593x; 6.6593x over previous
"""Sparse-attention Trainium2 kernel (8 NeuronCores, SPMD, single launch).

Sharding: core = b*4 + q handles batch b, image rows [28q, 28q+28) (3136 pixels).
The tiny 7x7 / 16x16 average pools (and the k = wk @ ctx_pool projection) are
done host-side in numpy -- they are reductions that would otherwise require a
second device launch plus a 44MB ctx upload.  The device kernel computes, per
core: q-projection, sparse top-32 attention, omega softmax, value aggregation
and the output projection, all in bf16 with fp32 PSUM accumulation.  The
residual (+ x) and projection bias are added on host in fp32.

All device I/O is bf16 to halve PJRT-tunnel traffic; dispatch uses a cached
compiled executable with device-resident (content-checked) weight buffers and
on-device-created donated zero output buffers.
"""
import sys
sys.path.insert(0, "/opt/trn_rl_repo")
import time
import numpy as np
import ml_dtypes

import concourse.bacc as bacc
import concourse.mybir as mybir
from concourse.tile import TileContext

F32 = mybir.dt.float32
BF16 = mybir.dt.bfloat16
F8 = mybir.dt.float8e4
Copy = mybir.ActivationFunctionType.Copy
Exp = mybir.ActivationFunctionType.Exp
X = mybir.AxisListType.X
ALU = mybir.AluOpType

NP_BF16 = ml_dtypes.bfloat16
NP_F8 = ml_dtypes.float8_e4m3

B, C, CC, H, W = 2, 768, 384, 112, 112
G, HD = 12, 64
S2, K2 = 49, 256
N = 28 * 112          # pixels per core
NW = 448              # window width (7 windows)
NSUB = 112            # stationary subtile width (4 per window)
NEG_INF = -1e30
NCORES = 8

X_DT = BF16           # dtype of the x upload (BF16 or F8)
OUT_DT = BF16         # dtype of the out download (BF16 or F8)

LAST_LAUNCH_NS = 0    # wall time of the device-facing span of the last call


def _np_dt(dt):
    return np.dtype(mybir.dt.np(dt))


def _new_nc():
    return bacc.Bacc("TRN2", target_bir_lowering=False, debug=False,
                     enable_asserts=False, num_devices=NCORES)


def _build_kernel(x_dt, out_dt):
    nc = _new_nc()
    xb = nc.dram_tensor("xb", [C, N], x_dt, kind="ExternalInput")
    wqT = nc.dram_tensor("wqT", [C, C], BF16, kind="ExternalInput")
    pwT = nc.dram_tensor("pwT", [C, C], BF16, kind="ExternalInput")
    wdp = nc.dram_tensor("wdp", [128, K2], BF16, kind="ExternalInput")
    k2s = nc.dram_tensor("k2s", [6 * 128, 2 * S2], BF16, kind="ExternalInput")
    va = nc.dram_tensor("va", [256, G * 65], BF16, kind="ExternalInput")
    ident = nc.dram_tensor("ident", [NSUB, NSUB], BF16, kind="ExternalInput")
    sel2 = nc.dram_tensor("sel2", [2, 128], BF16, kind="ExternalInput")
    ob = nc.dram_tensor("ob", [C, N], out_dt, kind="ExternalOutput")

    with TileContext(nc) as tc:
        with tc.tile_pool(name="wts", bufs=1) as wts, \
             tc.tile_pool(name="sb", bufs=2) as sb, \
             tc.tile_pool(name="ps", bufs=7, space="PSUM") as ps:

            def load(dram, shape, tag):
                t = wts.tile(shape, BF16, tag=tag)
                nc.sync.dma_start(t[:], dram)
                return t

            # persistent weights (already bf16 on host)
            wq_sb = [load(wqT[128 * t:128 * (t + 1), :], [128, C], f"wq{t}")
                     for t in range(6)]
            pw_sb = [load(pwT[128 * t:128 * (t + 1), :], [128, C], f"pw{t}")
                     for t in range(6)]
            wd_sb = load(wdp[:], [128, K2], "wd")
            wd_lo = wd_sb[0:S2, :]
            wd_hi = wd_sb[64:64 + S2, :]
            k2_sb = [load(k2s[128 * o:128 * (o + 1), :], [128, 2 * S2], f"k2_{o}")
                     for o in range(6)]
            va_sb = []
            for kh in range(2):
                vt = wts.tile([128, G, 65], BF16, tag=f"va{kh}")
                nc.sync.dma_start(vt[:].rearrange("p g a -> p (g a)"),
                                  va[128 * kh:128 * (kh + 1), :])
                va_sb.append(vt)
            id_sb = load(ident[:], [NSUB, NSUB], "id")
            s2_sb = load(sel2[:], [2, 128], "s2")

            for w in range(7):
                cw = slice(NW * w, NW * (w + 1))
                x_bf = []
                for t in range(6):
                    if x_dt == BF16:
                        xt = sb.tile([128, NW], BF16, tag=f"xb{t}")
                        nc.sync.dma_start(xt[:], xb[128 * t:128 * (t + 1), cw])
                    else:
                        x8 = sb.tile([128, NW], x_dt, tag=f"x8{t}")
                        nc.sync.dma_start(x8[:], xb[128 * t:128 * (t + 1), cw])
                        xt = sb.tile([128, NW], BF16, tag=f"xb{t}")
                        nc.vector.tensor_copy(xt[:], x8[:])
                    x_bf.append(xt)

                # q projection (scaled by hd^-0.5 on eviction)
                q_bf = []
                for o in range(6):
                    pq = ps.tile([128, NW], F32, tag="ps")
                    for ci in range(6):
                        nc.tensor.matmul(pq[:], wq_sb[ci][:, 128 * o:128 * (o + 1)],
                                         x_bf[ci][:], start=(ci == 0), stop=(ci == 5))
                    qb = sb.tile([128, NW], BF16, tag=f"q{o}")
                    nc.scalar.activation(qb[:], pq[:], Copy, scale=float(HD) ** -0.5)
                    q_bf.append(qb)

                oa_bf = []
                for t in range(6):
                    # attention scores for head pair t: [112, 4, 2, 49]
                    pa = ps.tile([NSUB, 4, 2 * S2], F32, tag="ps")
                    for s in range(4):
                        nc.tensor.matmul(pa[:, s, :],
                                         q_bf[t][:, NSUB * s:NSUB * (s + 1)],
                                         k2_sb[t][:])
                    at = sb.tile([NSUB, 4, 2, S2], F32, tag="at")
                    nc.scalar.activation(at[:], pa[:].rearrange("p a (b s) -> p a b s", s=S2), Copy)

                    # top-32 of 49: find 17th-smallest threshold via negated max8
                    ng = sb.tile([NSUB, 4, 2, S2], F32, tag="ng")
                    nc.vector.tensor_scalar_mul(ng[:], at[:], -1.0)
                    for s in range(4):
                        for hh in range(2):
                            m8 = sb.tile([NSUB, 8], F32, tag="m8")
                            sl = ng[:, s, hh, :]
                            nc.vector.max(out=m8[:], in_=sl)
                            nc.vector.match_replace(out=sl, in_to_replace=m8[:],
                                                    in_values=sl, imm_value=NEG_INF)
                            m8b = sb.tile([NSUB, 8], F32, tag="m8")
                            nc.vector.max(out=m8b[:], in_=sl)
                            nc.vector.match_replace(out=sl, in_to_replace=m8b[:],
                                                    in_values=sl, imm_value=NEG_INF)
                    tau = sb.tile([NSUB, 8], F32, tag="tau")
                    nc.vector.tensor_reduce(tau[:], ng[:].rearrange("p a b s -> p (a b) s"),
                                            axis=X, op=ALU.max)
                    nthr = sb.tile([NSUB, 8], F32, tag="nthr")
                    nc.vector.tensor_scalar_mul(nthr[:], tau[:], -1.0)
                    msk = sb.tile([NSUB, 4, 2, S2], F32, tag="msk")
                    nc.vector.tensor_tensor(
                        out=msk[:].rearrange("p a b s -> p (a b) s"),
                        in0=at[:].rearrange("p a b s -> p (a b) s"),
                        in1=nthr[:].to_broadcast((NSUB, 8, S2)), op=ALU.is_gt)
                    # sparse (bf16) into zero-padded [112, 4, 2, 64] for transpose
                    sp = sb.tile([NSUB, 4, 2, 64], BF16, tag="sp")
                    nc.gpsimd.memset(sp[:], 0.0)
                    nc.vector.tensor_tensor(out=sp[:, :, :, 0:S2], in0=at[:], in1=msk[:],
                                            op=ALU.mult)

                    # transpose via identity matmul -> spT [128, 4, 112] bf16
                    pt = ps.tile([128, 4, NSUB], F32, tag="ps")
                    for s in range(4):
                        nc.tensor.matmul(pt[:, s, :],
                                         sp[:, s, :, :].rearrange("p a b -> p (a b)"),
                                         id_sb[:])
                    spT = sb.tile([128, 4, NSUB], BF16, tag="spT")
                    nc.scalar.activation(spT[:], pt[:], Copy)

                    # per-head: omega logits -> exp -> out-mm (with sum row)
                    pouts = []
                    for hh in range(2):
                        base = 64 * hh
                        rhs = spT[base:base + S2, :, :].rearrange("p a b -> p (a b)")
                        po = ps.tile([65, NW], F32, tag="ps")
                        wd_use = wd_lo if hh == 0 else wd_hi
                        for kh in range(2):
                            pm = ps.tile([128, NW], F32, tag="ps")
                            nc.tensor.matmul(pm[:], wd_use[:, 128 * kh:128 * (kh + 1)], rhs)
                            ex = sb.tile([128, NW], BF16, tag="ex")
                            nc.scalar.activation(ex[:], pm[:], Exp)
                            nc.tensor.matmul(po[:], va_sb[kh][:, 2 * t + hh, :], ex[:],
                                             start=(kh == 0), stop=(kh == 1))
                        pouts.append(po)

                    # softmax denominators for the pair -> recip -> replicate
                    sm = sb.tile([2, NW], F32, tag="sm")
                    nc.scalar.activation(sm[0:1, :], pouts[0][64:65, :], Copy)
                    smt = sb.tile([1, NW], F32, tag="smt")
                    nc.scalar.activation(smt[0:1, :], pouts[1][64:65, :], Copy)
                    nc.sync.dma_start(sm[1:2, :], smt[0:1, :])
                    rc = sb.tile([2, NW], F32, tag="rc")
                    nc.vector.reciprocal(rc[:], sm[:])
                    rcb = sb.tile([2, NW], BF16, tag="rcb")
                    nc.vector.tensor_copy(rcb[:], rc[:])
                    pr = ps.tile([128, NW], F32, tag="ps")
                    nc.tensor.matmul(pr[:], s2_sb[:], rcb[:])
                    rep = sb.tile([128, NW], F32, tag="rep")
                    nc.scalar.activation(rep[:], pr[:], Copy)

                    oa = sb.tile([128, NW], BF16, tag=f"oa{t}")
                    for hh in range(2):
                        nc.vector.tensor_tensor(out=oa[64 * hh:64 * (hh + 1), :],
                                                in0=pouts[hh][0:64, :],
                                                in1=rep[64 * hh:64 * (hh + 1), :],
                                                op=ALU.mult)
                    oa_bf.append(oa)

                # output projection (bias + residual are added on host)
                for o in range(6):
                    py = ps.tile([128, NW], F32, tag="ps")
                    for ci in range(6):
                        nc.tensor.matmul(py[:], pw_sb[ci][:, 128 * o:128 * (o + 1)],
                                         oa_bf[ci][:], start=(ci == 0), stop=(ci == 5))
                    y = sb.tile([128, NW], out_dt, tag=f"y{o}")
                    nc.scalar.activation(y[:], py[:], Copy)
                    nc.sync.dma_start(ob[128 * o:128 * (o + 1), cw], y[:])
    nc.compile()
    return nc


class _Launcher:
    """Cached PJRT dispatcher mirroring bass2jax.run_bass_via_pjrt, with
    device-resident input caching, on-device zero output buffers, and
    async d2h fetch."""

    def __init__(self, nc):
        import jax
        from jax.sharding import Mesh, PartitionSpec, NamedSharding
        from jax.experimental.shard_map import shard_map
        from concourse.bass2jax import (_bass_exec_p, partition_id_tensor,
                                        install_neuronx_cc_hook)
        install_neuronx_cc_hook()
        self.jax = jax
        self.nc = nc
        if nc.dbg_addr is not None and nc.dbg_callbacks:
            raise RuntimeError("dbg_callbacks unsupported")

        partition_name = nc.partition_id_tensor.name if nc.partition_id_tensor else None
        in_names, out_names, out_avals = [], [], []
        for alloc in nc.m.functions[0].allocations:
            if not isinstance(alloc, mybir.MemoryLocationSet):
                continue
            name = alloc.memorylocations[0].name
            if alloc.kind == "ExternalInput":
                if name != partition_name:
                    in_names.append(name)
            elif alloc.kind == "ExternalOutput":
                shape = tuple(alloc.tensor_shape)
                dtype = mybir.dt.np(alloc.dtype)
                out_names.append(name)
                out_avals.append(jax.core.ShapedArray(shape, dtype))
        self.in_names = list(in_names)
        self.out_names = out_names
        self.out_avals = out_avals
        n_params = len(in_names)
        n_outs = len(out_avals)
        in_names_all = in_names + out_names + ([partition_name] if partition_name else [])
        donate = tuple(range(n_params, n_params + n_outs))

        def _body(*args):
            operands = list(args)
            if partition_name is not None:
                operands.append(partition_id_tensor())
            outs = _bass_exec_p.bind(
                *operands,
                out_avals=tuple(out_avals),
                in_names=tuple(in_names_all),
                out_names=tuple(out_names),
                lowering_input_output_aliases=(),
                sim_require_finite=True,
                sim_require_nnan=True,
                nc=nc,
            )
            return tuple(outs)

        devices = jax.devices()[:NCORES]
        assert len(devices) == NCORES
        mesh = Mesh(np.asarray(devices), ("core",))
        self.sh = NamedSharding(mesh, PartitionSpec("core"))
        in_specs = (PartitionSpec("core"),) * (n_params + n_outs)
        out_specs = (PartitionSpec("core"),) * n_outs
        self.sharded = jax.jit(
            shard_map(_body, mesh=mesh, in_specs=in_specs, out_specs=out_specs,
                      check_rep=False),
            donate_argnums=donate, keep_unused=True,
            in_shardings=(self.sh,) * (n_params + n_outs),
        )
        import jax.numpy as jnp
        import functools
        self.zeros_makers = []
        for av in out_avals:
            gshape = (NCORES * av.shape[0], *av.shape[1:])
            self.zeros_makers.append(jax.jit(
                functools.partial(jnp.zeros, gshape, av.dtype),
                out_shardings=self.sh))
        self.dev_cache = {}

    def submit(self, in_maps):
        """Upload (or reuse cached) inputs and dispatch; returns out arrays."""
        jax = self.jax
        if self.nc.dbg_addr is not None:
            z = np.zeros((1, 2), np.uint32)
            in_maps = [{**m, self.nc.dbg_addr.name: z} for m in in_maps]
        dev_args = []
        for name in self.in_names:
            parts = [np.ascontiguousarray(np.asarray(m[name])) for m in in_maps]
            cached = self.dev_cache.get(name)
            if cached is not None and all(
                    np.array_equal(parts[c], cached[1][c]) for c in range(NCORES)):
                dev_args.append(cached[0])
            else:
                conc = np.concatenate(parts, axis=0)
                darr = jax.device_put(conc, self.sh)
                self.dev_cache[name] = (darr, parts)
                dev_args.append(darr)
        zeros = [zm() for zm in self.zeros_makers]
        outs = self.sharded(*dev_args, *zeros)
        return outs

    def collect(self, outs):
        """Fetch all output shards to host; returns per-core dicts."""
        for o in outs:
            for s in o.addressable_shards:
                s.data.copy_to_host_async()
        results = [dict() for _ in range(NCORES)]
        for j, name in enumerate(self.out_names):
            shards = sorted(outs[j].addressable_shards,
                            key=lambda s: s.index[0].start or 0)
            assert len(shards) == NCORES
            for c, s in enumerate(shards):
                results[c][name] = np.asarray(s.data)
        return results

    def run(self, in_maps):
        return self.collect(self.submit(in_maps))


_CACHE = {}


def _get_launcher():
    key = ("launcher", str(X_DT), str(OUT_DT))
    if key not in _CACHE:
        nc = _build_kernel(X_DT, OUT_DT)
        _CACHE[key] = _Launcher(nc)
    return _CACHE[key]


def _host_prep(x, ctx, wq, wk, wd, proj_w, proj_b):
    """Build per-core input maps (pooling + weight packing in numpy)."""
    x_np = _np_dt(X_DT)

    # pooled ctx -> k -> block-diag head-pair layout k2 [B][768, 98]
    cp = ctx.reshape(B, CC, 7, 16, 7, 16).sum(5).sum(3) * (1.0 / 256.0)
    k2s = []
    for b in range(B):
        k = wk @ cp[b].reshape(CC, S2)                       # [768, 49]
        k2 = np.zeros((6, 128, 2 * S2), np.float32)
        for o in range(6):
            k2[o, 0:64, 0:S2] = k[128 * o:128 * o + 64]
            k2[o, 64:128, S2:2 * S2] = k[128 * o + 64:128 * (o + 1)]
        k2s.append(np.ascontiguousarray(
            k2.reshape(6 * 128, 2 * S2).astype(NP_BF16)))

    # pooled x -> v -> augmented value matrix va [B][256, 12*65]
    v = x.reshape(B, C, 16, 7, 16, 7).sum(5).sum(3) * (1.0 / 49.0)  # (B,C,16,16)
    vas = []
    for b in range(B):
        vt = v[b].reshape(C, K2).T                           # [256, 768]
        vab = np.ones((K2, G, 65), np.float32)
        vab[:, :, 0:64] = vt.reshape(K2, G, 64)
        vas.append(np.ascontiguousarray(
            vab.reshape(K2, G * 65).astype(NP_BF16)))

    wqT_bf = np.ascontiguousarray(wq.T.astype(NP_BF16))
    pwT_bf = np.ascontiguousarray(proj_w.T.astype(NP_BF16))
    wdp = np.zeros((128, K2), np.float32)
    wdT = wd.T                                               # [49, 256]
    wdp[0:S2, :] = wdT
    wdp[64:64 + S2, :] = wdT
    wdp_bf = wdp.astype(NP_BF16)
    id_bf = np.eye(NSUB, dtype=NP_BF16)
    sel2 = np.zeros((2, 128), np.float32)
    sel2[0, :64] = 1.0
    sel2[1, 64:] = 1.0
    sel2_bf = sel2.astype(NP_BF16)

    # per-core x slices, cast in one strided pass: (B,4,C,28,112)
    xc = np.ascontiguousarray(
        x.reshape(B, C, 4, 28, W).transpose(0, 2, 1, 3, 4).astype(x_np)
    ).reshape(NCORES, C, N)

    in_maps = []
    for core in range(NCORES):
        b = core // 4
        in_maps.append({
            "xb": xc[core], "wqT": wqT_bf, "pwT": pwT_bf, "wdp": wdp_bf,
            "k2s": k2s[b], "va": vas[b], "ident": id_bf, "sel2": sel2_bf,
        })
    return in_maps


def kernel(x, ctx, wq, wk, wd, proj_w, proj_b):
    global LAST_LAUNCH_NS
    x = np.ascontiguousarray(np.asarray(x, dtype=np.float32))
    ctx = np.ascontiguousarray(np.asarray(ctx, dtype=np.float32))
    wq = np.asarray(wq, dtype=np.float32)
    wk = np.asarray(wk, dtype=np.float32)
    wd = np.asarray(wd, dtype=np.float32)
    proj_w = np.asarray(proj_w, dtype=np.float32)
    proj_b = np.asarray(proj_b, dtype=np.float32)

    in_maps = _host_prep(x, ctx, wq, wk, wd, proj_w, proj_b)
    ln = _get_launcher()

    t0 = time.time()
    outs = ln.submit(in_maps)
    # overlap: residual + bias in fp32 while the device runs
    y = x + proj_b[None, :, None, None]
    results = ln.collect(outs)
    LAST_LAUNCH_NS = int((time.time() - t0) * 1e9)

    yv = y.reshape(B, C, 4, 28, W)
    for core in range(NCORES):
        b, q = core // 4, core % 4
        yv[b, :, q] += results[core]["ob"].reshape(C, 28, W).astype(np.float32)
    return y
